# revision 2
# baseline (speedup 1.0000x reference)
"""DeepPheno model kernel for 8 TRN2 NeuronCores.

Computation (reference):
    h    = gelu(gos @ W1 + b1)                     (B, HID)     erf-gelu
    x    = concat([h, exp_x], 1)                   (B, HID+EXP)
    flat = sigmoid(x @ W2 + b2)                    (B, C)
    out  = max_i flat[b, j] * M[i, j]              (B, C)

Since flat = sigmoid(..) > 0, the max-pool factorizes exactly:
    out[b, j] = flat[b, j] * max_i M[i, j]
(multiplying by a positive scalar is monotone, so the max over i is attained
at argmax_i M[i, j] on both sides and the products round identically).

Sharding over 8 cores (SPMD, all differences live in the shard data):
  - matmul1 split by HID columns: core c owns hid rows [192c, 192(c+1)) of
    h.T (HID padded 1500 -> 1536 with zero W1 columns, gelu(0)=0).
  - AllGather of the 8 gelu'd (192, 64) chunks -> every core holds the full
    x.T contraction operand for matmul2.
  - matmul2 / b2 / hpo colmax / output split by class columns: core c owns
    classes [256c, 256(c+1)).
Weight tensors are read by exactly one core; only gos (2.5MB) is replicated.

All device tensors are host-prepacked into SBUF-image layout (128, free) so
every DMA moves long contiguous per-partition rows.
"""

import numpy as np

import concourse.bacc as bacc
import concourse.mybir as mybir
import concourse.tile as tile
from concourse.bass_utils import run_bass_kernel_spmd

# Problem shape (hardcoded per contract)
B = 64
IN = 10000
EXP = 53
HID = 1500
C = 2048

NCORES = 8
HD = 192          # hid columns per core (1536 / 8)
HIDP = HD * NCORES
CD = C // NCORES  # 256 classes per core
KT1 = 79          # k tiles for matmul1: 79 * 128 = 10112 >= 10000
K1P = KT1 * 128
KT2 = 13          # k tiles for matmul2: 13 * 128 = 1664 = 1536 + 128
K2P = KT2 * 128

F32 = mybir.dt.float32

# DMA chunking (k-tile boundaries)
GOS_CHUNKS = [0, 20, 40, 60, KT1]
W1A_CHUNKS = [0, 16, 32, 48, 64, KT1]   # m-block 0 (128 cols)
W1B_CHUNKS = [0, 27, 53, KT1]           # m-block 1 (64 cols)


def _build_nc():
    nc = bacc.Bacc(
        "TRN2",
        target_bir_lowering=False,
        debug=False,
        enable_asserts=False,
        num_devices=NCORES,
    )

    # External I/O, all in SBUF-image layout (128, free)
    gos_d = nc.dram_tensor("gos_img", [128, KT1 * B], F32, kind="ExternalInput")
    w1a_d = nc.dram_tensor("w1a_img", [128, KT1 * 128], F32, kind="ExternalInput")
    w1b_d = nc.dram_tensor("w1b_img", [128, KT1 * 64], F32, kind="ExternalInput")
    w2_d = nc.dram_tensor("w2_img", [128, 2 * KT2 * 128], F32, kind="ExternalInput")
    exp_d = nc.dram_tensor("exp_img", [128, B], F32, kind="ExternalInput")
    mt_d = nc.dram_tensor("mt_img", [128, 2 * C], F32, kind="ExternalInput")
    b1_d = nc.dram_tensor("b1_img", [128, 2], F32, kind="ExternalInput")
    b2_d = nc.dram_tensor("b2_img", [128, 2], F32, kind="ExternalInput")
    out_d = nc.dram_tensor("out_img", [128, 2 * B], F32, kind="ExternalOutput")

    with tile.TileContext(nc) as tc:
        with (
            tc.tile_pool(name="persist", bufs=1) as pp,
            tc.tile_pool(name="small", bufs=1) as sp,
            tc.tile_pool(name="psum", bufs=1, space="PSUM") as psp,
            tc.tile_pool(name="dram", bufs=1, space="DRAM") as dp,
        ):
            # --- small loads first (keep them off the big-DMA tail) ---
            b1_sb = sp.tile([128, 2], F32, tag="b1")
            nc.sync.dma_start(out=b1_sb[:, :], in_=b1_d[:, :])
            b2_sb = sp.tile([128, 2], F32, tag="b2")
            nc.sync.dma_start(out=b2_sb[:, :], in_=b2_d[:, :])
            exp_sb = sp.tile([128, B], F32, tag="expt")
            nc.sync.dma_start(out=exp_sb[:, :], in_=exp_d[:, :])

            # --- streaming operand gos.T, chunked ---
            gos_sb = pp.tile([128, KT1 * B], F32, tag="gos")
            for a, b in zip(GOS_CHUNKS[:-1], GOS_CHUNKS[1:]):
                nc.sync.dma_start(
                    out=gos_sb[:, a * B : b * B], in_=gos_d[:, a * B : b * B]
                )

            # --- W1 shard, m-block-major, chunked ---
            w1a_sb = pp.tile([128, KT1 * 128], F32, tag="w1a")
            for a, b in zip(W1A_CHUNKS[:-1], W1A_CHUNKS[1:]):
                nc.sync.dma_start(
                    out=w1a_sb[:, a * 128 : b * 128], in_=w1a_d[:, a * 128 : b * 128]
                )
            w1b_sb = pp.tile([128, KT1 * 64], F32, tag="w1b")
            for a, b in zip(W1B_CHUNKS[:-1], W1B_CHUNKS[1:]):
                nc.sync.dma_start(
                    out=w1b_sb[:, a * 64 : b * 64], in_=w1b_d[:, a * 64 : b * 64]
                )

            # --- W2 shard (both m-blocks), then hpo M.T shard ---
            w2_sb = pp.tile([128, 2 * KT2 * 128], F32, tag="w2")
            nc.sync.dma_start(
                out=w2_sb[:, : KT2 * 128], in_=w2_d[:, : KT2 * 128]
            )
            nc.sync.dma_start(
                out=w2_sb[:, KT2 * 128 :], in_=w2_d[:, KT2 * 128 :]
            )
            mt_sb = pp.tile([128, 2 * C], F32, tag="mt")
            nc.sync.dma_start(out=mt_sb[:, :C], in_=mt_d[:, :C])
            nc.sync.dma_start(out=mt_sb[:, C:], in_=mt_d[:, C:])

            # --- matmul1: h.T chunk = W1c.T @ gos.T, k accumulated in PSUM ---
            ph0 = psp.tile([128, B], F32, tag="ph0")
            for n in range(KT1):
                nc.tensor.matmul(
                    ph0[:, :],
                    lhsT=w1a_sb[:, n * 128 : (n + 1) * 128],
                    rhs=gos_sb[:, n * B : (n + 1) * B],
                    start=(n == 0),
                    stop=(n == KT1 - 1),
                )
            ph1 = psp.tile([64, B], F32, tag="ph1")
            for n in range(KT1):
                nc.tensor.matmul(
                    ph1[:, :],
                    lhsT=w1b_sb[:, n * 64 : (n + 1) * 64],
                    rhs=gos_sb[:, n * B : (n + 1) * B],
                    start=(n == 0),
                    stop=(n == KT1 - 1),
                )

            # gelu(pre + b1)  (exact erf gelu)
            h0_sb = sp.tile([128, B], F32, tag="h0")
            nc.scalar.activation(
                h0_sb[:, :], ph0[:, :],
                mybir.ActivationFunctionType.Gelu, bias=b1_sb[:, 0:1],
            )
            h1_sb = sp.tile([64, B], F32, tag="h1")
            nc.scalar.activation(
                h1_sb[:, :], ph1[:, :],
                mybir.ActivationFunctionType.Gelu, bias=b1_sb[0:64, 1:2],
            )

            # --- AllGather the (192, B) chunk -> full (1536, B) h.T ---
            ag_in = dp.tile([HD, B], F32, tag="ag_in")
            nc.sync.dma_start(out=ag_in[0:128, :], in_=h0_sb[:, :])
            nc.sync.dma_start(out=ag_in[128:HD, :], in_=h1_sb[:, :])
            ag_out = dp.tile([HIDP, B], F32, tag="ag_out")
            nc.gpsimd.collective_compute(
                "AllGather",
                mybir.AluOpType.bypass,
                replica_groups=[list(range(NCORES))],
                ins=[ag_in[:, :].opt()],
                outs=[ag_out[:, :].opt()],
            )

            # full x.T (k on partitions): 12 tiles from the gather + exp tile
            x_sb = pp.tile([128, (KT2 - 1) * B], F32, tag="xsb")
            nc.sync.dma_start(
                out=x_sb[:, :].rearrange("p (n m) -> p n m", n=KT2 - 1),
                in_=ag_out[:, :].rearrange("(n p) m -> p n m", p=128),
            )

            # --- matmul2: flat.T chunk = W2c.T @ x.T ---
            pf0 = psp.tile([128, B], F32, tag="pf0")
            pf1 = psp.tile([128, B], F32, tag="pf1")
            for mb, pf in ((0, pf0), (1, pf1)):
                base = mb * KT2 * 128
                for t in range(KT2):
                    rhs = (
                        x_sb[:, t * B : (t + 1) * B]
                        if t < KT2 - 1
                        else exp_sb[:, :]
                    )
                    nc.tensor.matmul(
                        pf[:, :],
                        lhsT=w2_sb[:, base + t * 128 : base + (t + 1) * 128],
                        rhs=rhs,
                        start=(t == 0),
                        stop=(t == KT2 - 1),
                    )

            # sigmoid(pre + b2)
            f0_sb = sp.tile([128, B], F32, tag="f0")
            nc.scalar.activation(
                f0_sb[:, :], pf0[:, :],
                mybir.ActivationFunctionType.Sigmoid, bias=b2_sb[:, 0:1],
            )
            f1_sb = sp.tile([128, B], F32, tag="f1")
            nc.scalar.activation(
                f1_sb[:, :], pf1[:, :],
                mybir.ActivationFunctionType.Sigmoid, bias=b2_sb[:, 1:2],
            )

            # column max of M for our 256 classes (rows of M.T, free-axis reduce)
            cm_sb = sp.tile([128, 2], F32, tag="cm")
            nc.vector.reduce_max(
                cm_sb[:, 0:1], mt_sb[:, :C], axis=mybir.AxisListType.X
            )
            nc.vector.reduce_max(
                cm_sb[:, 1:2], mt_sb[:, C:], axis=mybir.AxisListType.X
            )

            # out.T chunk = flat.T * colmax (per-partition broadcast)
            o_sb = sp.tile([128, 2 * B], F32, tag="osb")
            nc.vector.tensor_scalar_mul(o_sb[:, :B], f0_sb[:, :], cm_sb[:, 0:1])
            nc.vector.tensor_scalar_mul(o_sb[:, B:], f1_sb[:, :], cm_sb[:, 1:2])
            nc.sync.dma_start(out=out_d[:, :], in_=o_sb[:, :])

    nc.compile()
    return nc


_NC_CACHE = None


def _get_nc():
    global _NC_CACHE
    if _NC_CACHE is None:
        _NC_CACHE = _build_nc()
    return _NC_CACHE


def _tile_img(arr2d, ktiles):
    """(ktiles*128, m) -> SBUF image (128, ktiles*m), k-tile-major free dim."""
    k, m = arr2d.shape
    assert k == ktiles * 128
    return np.ascontiguousarray(
        arr2d.reshape(ktiles, 128, m).transpose(1, 0, 2).reshape(128, ktiles * m)
    )


def _prep_inputs(gos, exp_x, W1, b1, W2, b2, hpo_matrix):
    f = np.float32
    gos = np.asarray(gos, f)
    exp_x = np.asarray(exp_x, f)
    W1 = np.asarray(W1, f)
    b1 = np.asarray(b1, f)
    W2 = np.asarray(W2, f)
    b2 = np.asarray(b2, f)
    M = np.asarray(hpo_matrix, f)

    # gos.T padded to K1P rows, shared across cores
    gosT = np.zeros((K1P, B), f)
    gosT[:IN] = gos.T
    gos_img = _tile_img(gosT, KT1)

    # exp_x.T padded to one 128-row k-tile, shared
    expT = np.zeros((128, B), f)
    expT[:EXP] = exp_x.T
    exp_img = np.ascontiguousarray(expT)

    # W1 padded to (K1P, HIDP)
    W1p = np.zeros((K1P, HIDP), f)
    W1p[:IN, :HID] = W1
    b1p = np.zeros((HIDP,), f)
    b1p[:HID] = b1

    # W2 rows remapped to x.T layout [h(0:1536) | exp(1536:1589) | 0 pad]
    W2p = np.zeros((K2P, C), f)
    W2p[:HID] = W2[:HID]
    W2p[HIDP : HIDP + EXP] = W2[HID:]

    in_maps = []
    for c in range(NCORES):
        h0, h1 = HD * c, HD * (c + 1)
        c0, c1 = CD * c, CD * (c + 1)

        w1a_img = _tile_img(W1p[:, h0 : h0 + 128], KT1)
        w1b_img = _tile_img(W1p[:, h0 + 128 : h1], KT1)

        w2c = W2p[:, c0:c1]
        w2_img = np.concatenate(
            [_tile_img(w2c[:, :128], KT2), _tile_img(w2c[:, 128:], KT2)], axis=1
        )

        mt = np.ascontiguousarray(M[:, c0:c1].T)  # (256, 2048)
        mt_img = np.concatenate([mt[:128], mt[128:]], axis=1)  # (128, 2C)

        b1_img = np.zeros((128, 2), f)
        b1_img[:, 0] = b1p[h0 : h0 + 128]
        b1_img[:64, 1] = b1p[h0 + 128 : h1]
        b2_img = np.zeros((128, 2), f)
        b2_img[:, 0] = b2[c0 : c0 + 128]
        b2_img[:, 1] = b2[c0 + 128 : c1]

        in_maps.append(
            {
                "gos_img": gos_img,
                "w1a_img": np.ascontiguousarray(w1a_img),
                "w1b_img": np.ascontiguousarray(w1b_img),
                "w2_img": np.ascontiguousarray(w2_img),
                "exp_img": exp_img,
                "mt_img": np.ascontiguousarray(mt_img),
                "b1_img": b1_img,
                "b2_img": b2_img,
            }
        )
    return in_maps


def _assemble_output(results):
    cols = []
    for r in results:
        o = r["out_img"]  # (128, 2B): [p, t*B + b] = flat.T[t*128+p, b] * cm
        chunk = o.reshape(128, 2, B).transpose(1, 0, 2).reshape(CD, B)
        cols.append(chunk.T)  # (B, CD)
    return np.ascontiguousarray(np.concatenate(cols, axis=1))


def kernel(gos, exp_x, W1, b1, W2, b2, hpo_matrix, **kw):
    nc = _get_nc()
    in_maps = _prep_inputs(gos, exp_x, W1, b1, W2, b2, hpo_matrix)
    res = run_bass_kernel_spmd(nc, in_maps, core_ids=list(range(NCORES)))
    return _assemble_output(res.results)


# revision 3
# speedup vs baseline: 1.1895x; 1.1895x over previous
"""DeepPheno model kernel for 8 TRN2 NeuronCores.

Computation (reference):
    h    = gelu(gos @ W1 + b1)                     (B, HID)     erf-gelu
    x    = concat([h, exp_x], 1)                   (B, HID+EXP)
    flat = sigmoid(x @ W2 + b2)                    (B, C)
    out  = max_i flat[b, j] * M[i, j]              (B, C)

Since flat = sigmoid(..) > 0, the max-pool factorizes exactly:
    out[b, j] = flat[b, j] * max_i M[i, j]
(multiplying by a positive scalar is monotone, so the max over i is attained
at argmax_i M[i, j] on both sides and the products round identically).

Sharding over 8 cores (SPMD, all differences live in the shard data):
  - matmul1 split by HID columns: core c owns hid rows [192c, 192(c+1)) of
    h.T (HID padded 1500 -> 1536 with zero W1 columns, gelu(0)=0).
  - AllGather of the 8 gelu'd (192, 64) chunks -> every core holds the full
    x.T contraction operand for matmul2.
  - matmul2 / b2 / hpo colmax / output split by class columns: core c owns
    classes [256c, 256(c+1)).
Weight tensors are read by exactly one core; only gos (2.5MB) is replicated.

Matmuls run in fp16 (fp32 matmul is 4 cycles/row and never HAM-warms; fp16 is
1 cycle/row): DMA stays fp32 (exact bytes), operands are cast to fp16 on
device overlapped with the DMA stream. PSUM accumulation, bias-add,
activations, colmax and the final multiply stay fp32.

All device tensors are host-prepacked into SBUF-image layout (128, free) so
every DMA moves long contiguous per-partition rows.
"""

import numpy as np

import concourse.bacc as bacc
import concourse.mybir as mybir
import concourse.tile as tile
from concourse.bass_utils import run_bass_kernel_spmd

# Problem shape (hardcoded per contract)
B = 64
IN = 10000
EXP = 53
HID = 1500
C = 2048

NCORES = 8
HD = 192          # hid columns per core (1536 / 8)
HIDP = HD * NCORES
CD = C // NCORES  # 256 classes per core
KT1 = 79          # k tiles for matmul1: 79 * 128 = 10112 >= 10000
K1P = KT1 * 128
KT2 = 13          # k tiles for matmul2: 13 * 128 = 1664 = 1536 + 128
K2P = KT2 * 128

F32 = mybir.dt.float32
F16 = mybir.dt.float16

# DMA chunking (k-tile boundaries)
GOS_CHUNKS = [0, 20, 40, 60, KT1]
W1A_CHUNKS = [0, 16, 32, 48, 64, KT1]   # m-block 0 (128 cols)
W1B_CHUNKS = [0, 27, 53, KT1]           # m-block 1 (64 cols)


def _build_nc():
    nc = bacc.Bacc(
        "TRN2",
        target_bir_lowering=False,
        debug=False,
        enable_asserts=False,
        num_devices=NCORES,
    )

    # External I/O, all in SBUF-image layout (128, free)
    gos_d = nc.dram_tensor("gos_img", [128, KT1 * B], F32, kind="ExternalInput")
    w1a_d = nc.dram_tensor("w1a_img", [128, KT1 * 128], F32, kind="ExternalInput")
    w1b_d = nc.dram_tensor("w1b_img", [128, KT1 * 64], F32, kind="ExternalInput")
    w2_d = nc.dram_tensor("w2_img", [128, 2 * KT2 * 128], F32, kind="ExternalInput")
    exp_d = nc.dram_tensor("exp_img", [128, B], F32, kind="ExternalInput")
    mt_d = nc.dram_tensor("mt_img", [128, 2 * C], F32, kind="ExternalInput")
    b1_d = nc.dram_tensor("b1_img", [128, 2], F32, kind="ExternalInput")
    b2_d = nc.dram_tensor("b2_img", [128, 2], F32, kind="ExternalInput")
    out_d = nc.dram_tensor("out_img", [128, 2 * B], F32, kind="ExternalOutput")

    with tile.TileContext(nc) as tc:
        with (
            tc.tile_pool(name="persist", bufs=1) as pp,
            tc.tile_pool(name="small", bufs=1) as sp,
            tc.tile_pool(name="psum", bufs=1, space="PSUM") as psp,
            tc.tile_pool(name="dram", bufs=1, space="DRAM") as dp,
        ):
            # --- small loads first (keep them off the big-DMA tail) ---
            b1_sb = sp.tile([128, 2], F32, tag="b1")
            nc.sync.dma_start(out=b1_sb[:, :], in_=b1_d[:, :])
            b2_sb = sp.tile([128, 2], F32, tag="b2")
            nc.sync.dma_start(out=b2_sb[:, :], in_=b2_d[:, :])
            exp_sb = sp.tile([128, B], F32, tag="expt")
            nc.sync.dma_start(out=exp_sb[:, :], in_=exp_d[:, :])
            exp16 = sp.tile([128, B], F16, tag="expt16")
            nc.gpsimd.tensor_copy(exp16[:, :], exp_sb[:, :])

            # --- streaming operand gos.T, chunked; cast fp16 on GpSimd ---
            gos_sb = pp.tile([128, KT1 * B], F32, tag="gos")
            gos16 = pp.tile([128, KT1 * B], F16, tag="gos16")
            for a, b in zip(GOS_CHUNKS[:-1], GOS_CHUNKS[1:]):
                sl = slice(a * B, b * B)
                nc.sync.dma_start(out=gos_sb[:, sl], in_=gos_d[:, sl])
                nc.gpsimd.tensor_copy(gos16[:, sl], gos_sb[:, sl])

            # --- W1 shard, m-block-major, chunked; cast on DVE/ACT alternating ---
            w1a_sb = pp.tile([128, KT1 * 128], F32, tag="w1a")
            w1a16 = pp.tile([128, KT1 * 128], F16, tag="w1a16")
            for i, (a, b) in enumerate(zip(W1A_CHUNKS[:-1], W1A_CHUNKS[1:])):
                sl = slice(a * 128, b * 128)
                nc.sync.dma_start(out=w1a_sb[:, sl], in_=w1a_d[:, sl])
                if i % 2 == 0:
                    nc.vector.tensor_copy(w1a16[:, sl], w1a_sb[:, sl])
                else:
                    nc.scalar.copy(w1a16[:, sl], w1a_sb[:, sl])
            w1b_sb = pp.tile([128, KT1 * 64], F32, tag="w1b")
            w1b16 = pp.tile([128, KT1 * 64], F16, tag="w1b16")
            for i, (a, b) in enumerate(zip(W1B_CHUNKS[:-1], W1B_CHUNKS[1:])):
                sl = slice(a * 64, b * 64)
                nc.sync.dma_start(out=w1b_sb[:, sl], in_=w1b_d[:, sl])
                if i % 2 == 0:
                    nc.vector.tensor_copy(w1b16[:, sl], w1b_sb[:, sl])
                else:
                    nc.scalar.copy(w1b16[:, sl], w1b_sb[:, sl])

            # --- W2 shard (both m-blocks), then hpo M.T shard ---
            w2_sb = pp.tile([128, 2 * KT2 * 128], F32, tag="w2")
            w2_16 = pp.tile([128, 2 * KT2 * 128], F16, tag="w2_16")
            for mb in range(2):
                sl = slice(mb * KT2 * 128, (mb + 1) * KT2 * 128)
                nc.sync.dma_start(out=w2_sb[:, sl], in_=w2_d[:, sl])
                nc.vector.tensor_copy(w2_16[:, sl], w2_sb[:, sl])
            mt_sb = pp.tile([128, 2 * C], F32, tag="mt")
            cm_sb = sp.tile([128, 2], F32, tag="cm")
            for mb in range(2):
                sl = slice(mb * C, (mb + 1) * C)
                nc.sync.dma_start(out=mt_sb[:, sl], in_=mt_d[:, sl])
                nc.vector.reduce_max(
                    cm_sb[:, mb : mb + 1], mt_sb[:, sl], axis=mybir.AxisListType.X
                )

            # --- matmul1: h.T chunk = W1c.T @ gos.T, k accumulated in PSUM ---
            ph0 = psp.tile([128, B], F32, tag="ph0")
            for n in range(KT1):
                nc.tensor.matmul(
                    ph0[:, :],
                    lhsT=w1a16[:, n * 128 : (n + 1) * 128],
                    rhs=gos16[:, n * B : (n + 1) * B],
                    start=(n == 0),
                    stop=(n == KT1 - 1),
                )
            ph1 = psp.tile([64, B], F32, tag="ph1")
            for n in range(KT1):
                nc.tensor.matmul(
                    ph1[:, :],
                    lhsT=w1b16[:, n * 64 : (n + 1) * 64],
                    rhs=gos16[:, n * B : (n + 1) * B],
                    start=(n == 0),
                    stop=(n == KT1 - 1),
                )

            # gelu(pre + b1) (exact erf gelu), output fp16 for the gather
            h0_sb = sp.tile([128, B], F16, tag="h0")
            nc.scalar.activation(
                h0_sb[:, :], ph0[:, :],
                mybir.ActivationFunctionType.Gelu, bias=b1_sb[:, 0:1],
            )
            h1_sb = sp.tile([64, B], F16, tag="h1")
            nc.scalar.activation(
                h1_sb[:, :], ph1[:, :],
                mybir.ActivationFunctionType.Gelu, bias=b1_sb[0:64, 1:2],
            )

            # --- AllGather the (192, B) fp16 chunk -> full (1536, B) h.T ---
            ag_in = dp.tile([HD, B], F16, tag="ag_in")
            nc.sync.dma_start(out=ag_in[0:128, :], in_=h0_sb[:, :])
            nc.sync.dma_start(out=ag_in[128:HD, :], in_=h1_sb[:, :])
            ag_out = dp.tile([HIDP, B], F16, tag="ag_out")
            nc.gpsimd.collective_compute(
                "AllGather",
                mybir.AluOpType.bypass,
                replica_groups=[list(range(NCORES))],
                ins=[ag_in[:, :].opt()],
                outs=[ag_out[:, :].opt()],
            )

            # full x.T (k on partitions): 12 tiles from the gather + exp tile
            x_sb = pp.tile([128, (KT2 - 1) * B], F16, tag="xsb")
            nc.sync.dma_start(
                out=x_sb[:, :].rearrange("p (n m) -> p n m", n=KT2 - 1),
                in_=ag_out[:, :].rearrange("(n p) m -> p n m", p=128),
            )

            # --- matmul2: flat.T chunk = W2c.T @ x.T ---
            pf0 = psp.tile([128, B], F32, tag="pf0")
            pf1 = psp.tile([128, B], F32, tag="pf1")
            for mb, pf in ((0, pf0), (1, pf1)):
                base = mb * KT2 * 128
                for t in range(KT2):
                    rhs = (
                        x_sb[:, t * B : (t + 1) * B]
                        if t < KT2 - 1
                        else exp16[:, :]
                    )
                    nc.tensor.matmul(
                        pf[:, :],
                        lhsT=w2_16[:, base + t * 128 : base + (t + 1) * 128],
                        rhs=rhs,
                        start=(t == 0),
                        stop=(t == KT2 - 1),
                    )

            # sigmoid(pre + b2)
            f0_sb = sp.tile([128, B], F32, tag="f0")
            nc.scalar.activation(
                f0_sb[:, :], pf0[:, :],
                mybir.ActivationFunctionType.Sigmoid, bias=b2_sb[:, 0:1],
            )
            f1_sb = sp.tile([128, B], F32, tag="f1")
            nc.scalar.activation(
                f1_sb[:, :], pf1[:, :],
                mybir.ActivationFunctionType.Sigmoid, bias=b2_sb[:, 1:2],
            )

            # out.T chunk = flat.T * colmax (per-partition broadcast)
            o_sb = sp.tile([128, 2 * B], F32, tag="osb")
            nc.vector.tensor_scalar_mul(o_sb[:, :B], f0_sb[:, :], cm_sb[:, 0:1])
            nc.vector.tensor_scalar_mul(o_sb[:, B:], f1_sb[:, :], cm_sb[:, 1:2])
            nc.sync.dma_start(out=out_d[:, :], in_=o_sb[:, :])

    nc.compile()
    return nc


_NC_CACHE = None


def _get_nc():
    global _NC_CACHE
    if _NC_CACHE is None:
        _NC_CACHE = _build_nc()
    return _NC_CACHE


def _tile_img(arr2d, ktiles):
    """(ktiles*128, m) -> SBUF image (128, ktiles*m), k-tile-major free dim."""
    k, m = arr2d.shape
    assert k == ktiles * 128
    return np.ascontiguousarray(
        arr2d.reshape(ktiles, 128, m).transpose(1, 0, 2).reshape(128, ktiles * m)
    )


def _prep_inputs(gos, exp_x, W1, b1, W2, b2, hpo_matrix):
    f = np.float32
    gos = np.asarray(gos, f)
    exp_x = np.asarray(exp_x, f)
    W1 = np.asarray(W1, f)
    b1 = np.asarray(b1, f)
    W2 = np.asarray(W2, f)
    b2 = np.asarray(b2, f)
    M = np.asarray(hpo_matrix, f)

    # gos.T padded to K1P rows, shared across cores
    gosT = np.zeros((K1P, B), f)
    gosT[:IN] = gos.T
    gos_img = _tile_img(gosT, KT1)

    # exp_x.T padded to one 128-row k-tile, shared
    expT = np.zeros((128, B), f)
    expT[:EXP] = exp_x.T
    exp_img = np.ascontiguousarray(expT)

    # W1 padded to (K1P, HIDP)
    W1p = np.zeros((K1P, HIDP), f)
    W1p[:IN, :HID] = W1
    b1p = np.zeros((HIDP,), f)
    b1p[:HID] = b1

    # W2 rows remapped to x.T layout [h(0:1536) | exp(1536:1589) | 0 pad]
    W2p = np.zeros((K2P, C), f)
    W2p[:HID] = W2[:HID]
    W2p[HIDP : HIDP + EXP] = W2[HID:]

    in_maps = []
    for c in range(NCORES):
        h0, h1 = HD * c, HD * (c + 1)
        c0, c1 = CD * c, CD * (c + 1)

        w1a_img = _tile_img(W1p[:, h0 : h0 + 128], KT1)
        w1b_img = _tile_img(W1p[:, h0 + 128 : h1], KT1)

        w2c = W2p[:, c0:c1]
        w2_img = np.concatenate(
            [_tile_img(w2c[:, :128], KT2), _tile_img(w2c[:, 128:], KT2)], axis=1
        )

        mt = np.ascontiguousarray(M[:, c0:c1].T)  # (256, 2048)
        mt_img = np.concatenate([mt[:128], mt[128:]], axis=1)  # (128, 2C)

        b1_img = np.zeros((128, 2), f)
        b1_img[:, 0] = b1p[h0 : h0 + 128]
        b1_img[:64, 1] = b1p[h0 + 128 : h1]
        b2_img = np.zeros((128, 2), f)
        b2_img[:, 0] = b2[c0 : c0 + 128]
        b2_img[:, 1] = b2[c0 + 128 : c1]

        in_maps.append(
            {
                "gos_img": gos_img,
                "w1a_img": np.ascontiguousarray(w1a_img),
                "w1b_img": np.ascontiguousarray(w1b_img),
                "w2_img": np.ascontiguousarray(w2_img),
                "exp_img": exp_img,
                "mt_img": np.ascontiguousarray(mt_img),
                "b1_img": b1_img,
                "b2_img": b2_img,
            }
        )
    return in_maps


def _assemble_output(results):
    cols = []
    for r in results:
        o = r["out_img"]  # (128, 2B): [p, t*B + b] = flat.T[t*128+p, b] * cm
        chunk = o.reshape(128, 2, B).transpose(1, 0, 2).reshape(CD, B)
        cols.append(chunk.T)  # (B, CD)
    return np.ascontiguousarray(np.concatenate(cols, axis=1))


def kernel(gos, exp_x, W1, b1, W2, b2, hpo_matrix, **kw):
    nc = _get_nc()
    in_maps = _prep_inputs(gos, exp_x, W1, b1, W2, b2, hpo_matrix)
    res = run_bass_kernel_spmd(nc, in_maps, core_ids=list(range(NCORES)))
    return _assemble_output(res.results)


# revision 5
# speedup vs baseline: 1.2811x; 1.0770x over previous
"""DeepPheno model kernel for 8 TRN2 NeuronCores.

Computation (reference):
    h    = gelu(gos @ W1 + b1)                     (B, HID)     erf-gelu
    x    = concat([h, exp_x], 1)                   (B, HID+EXP)
    flat = sigmoid(x @ W2 + b2)                    (B, C)
    out  = max_i flat[b, j] * M[i, j]              (B, C)

Since flat = sigmoid(..) > 0, the max-pool factorizes exactly:
    out[b, j] = flat[b, j] * max_i M[i, j]
(multiplying by a positive scalar is monotone, so the max over i is attained
at argmax_i M[i, j] on both sides and the products round identically).

Sharding over 8 cores (SPMD, all differences live in the shard data):
  - matmul1 split by HID columns: core c owns hid rows [192c, 192(c+1)) of
    h.T (HID padded 1500 -> 1536 with zero W1 columns, gelu(0)=0).
  - AllGather of the 8 gelu'd (192, 64) chunks -> every core holds the full
    x.T contraction operand for matmul2.
  - matmul2 / b2 / hpo colmax / output split by class columns: core c owns
    classes [256c, 256(c+1)).
Weight tensors are read by exactly one core; only gos (2.5MB) is replicated.

Matmuls run in fp16 (fp32 matmul is 4 cycles/row and never HAM-warms; fp16 is
1 cycle/row): DMA stays fp32 (exact bytes), operands are cast to fp16 on
device overlapped with the DMA stream. PSUM accumulation, bias-add,
activations, colmax and the final multiply stay fp32.

All device tensors are host-prepacked into SBUF-image layout (128, free) so
every DMA moves long contiguous per-partition rows.
"""

import numpy as np

import concourse.bacc as bacc
import concourse.mybir as mybir
import concourse.tile as tile
from concourse.bass_utils import run_bass_kernel_spmd

# Problem shape (hardcoded per contract)
B = 64
IN = 10000
EXP = 53
HID = 1500
C = 2048

NCORES = 8
HD = 192          # hid columns per core (1536 / 8)
HIDP = HD * NCORES
CD = C // NCORES  # 256 classes per core
KT1 = 79          # k tiles for matmul1: 79 * 128 = 10112 >= 10000
K1P = KT1 * 128
KT2 = 13          # k tiles for matmul2: 13 * 128 = 1664 = 1536 + 128
K2P = KT2 * 128

F32 = mybir.dt.float32
F16 = mybir.dt.float16

# DMA chunking (k-tile boundaries)
GOS_CHUNKS = [0, 20, 40, 60, KT1]
W1A_CHUNKS = [0, 16, 32, 48, 64, KT1]   # m-block 0 (128 cols)
W1B_CHUNKS = [0, 27, 53, KT1]           # m-block 1 (64 cols)


def _build_nc():
    nc = bacc.Bacc(
        "TRN2",
        target_bir_lowering=False,
        debug=False,
        enable_asserts=False,
        num_devices=NCORES,
    )

    # External I/O, all in SBUF-image layout (128, free)
    gos_d = nc.dram_tensor("gos_img", [128, KT1 * B], F32, kind="ExternalInput")
    w1a_d = nc.dram_tensor("w1a_img", [128, KT1 * 128], F32, kind="ExternalInput")
    w1b_d = nc.dram_tensor("w1b_img", [128, KT1 * 64], F32, kind="ExternalInput")
    w2_d = nc.dram_tensor("w2_img", [128, 2 * KT2 * 128], F32, kind="ExternalInput")
    exp_d = nc.dram_tensor("exp_img", [128, B], F32, kind="ExternalInput")
    mt_d = nc.dram_tensor("mt_img", [128, 2 * C], F32, kind="ExternalInput")
    b1_d = nc.dram_tensor("b1_img", [128, 2], F32, kind="ExternalInput")
    b2_d = nc.dram_tensor("b2_img", [128, 2], F32, kind="ExternalInput")
    out_d = nc.dram_tensor("out_img", [128, 2 * B], F32, kind="ExternalOutput")

    with tile.TileContext(nc) as tc:
        with (
            tc.tile_pool(name="persist", bufs=1) as pp,
            tc.tile_pool(name="small", bufs=1) as sp,
            tc.tile_pool(name="psum", bufs=1, space="PSUM") as psp,
            tc.tile_pool(name="dram", bufs=1, space="DRAM") as dp,
        ):
            # --- dummy tiny AllGather issued first: absorbs the ncfw entry
            # barrier (~30-50us) while the big DMAs stream, so the real
            # gather later pays only its own latency ---
            dumm_in = dp.tile([1, 8], F32, tag="dumm_in")
            dumm_out = dp.tile([NCORES, 8], F32, tag="dumm_out")
            dumm_sb = sp.tile([1, 8], F32, tag="dumm_sb")
            nc.vector.memset(dumm_sb[:, :], 0.0)
            nc.sync.dma_start(out=dumm_in[:, :], in_=dumm_sb[:, :])
            nc.gpsimd.collective_compute(
                "AllGather",
                mybir.AluOpType.bypass,
                replica_groups=[list(range(NCORES))],
                ins=[dumm_in[:, :].opt()],
                outs=[dumm_out[:, :].opt()],
            )

            # --- small loads first (keep them off the big-DMA tail) ---
            b1_sb = sp.tile([128, 2], F32, tag="b1")
            nc.sync.dma_start(out=b1_sb[:, :], in_=b1_d[:, :])
            b2_sb = sp.tile([128, 2], F32, tag="b2")
            nc.sync.dma_start(out=b2_sb[:, :], in_=b2_d[:, :])
            exp_sb = sp.tile([128, B], F32, tag="expt")
            nc.sync.dma_start(out=exp_sb[:, :], in_=exp_d[:, :])
            exp16 = sp.tile([128, B], F16, tag="expt16")
            nc.scalar.copy(exp16[:, :], exp_sb[:, :])

            # --- streaming operand gos.T, chunked; cast fp16 on DVE ---
            gos_sb = pp.tile([128, KT1 * B], F32, tag="gos")
            gos16 = pp.tile([128, KT1 * B], F16, tag="gos16")
            for a, b in zip(GOS_CHUNKS[:-1], GOS_CHUNKS[1:]):
                sl = slice(a * B, b * B)
                nc.sync.dma_start(out=gos_sb[:, sl], in_=gos_d[:, sl])
                nc.vector.tensor_copy(gos16[:, sl], gos_sb[:, sl])

            # --- W1 shard, m-block-major, chunked; cast on DVE/ACT alternating ---
            w1a_sb = pp.tile([128, KT1 * 128], F32, tag="w1a")
            w1a16 = pp.tile([128, KT1 * 128], F16, tag="w1a16")
            for i, (a, b) in enumerate(zip(W1A_CHUNKS[:-1], W1A_CHUNKS[1:])):
                sl = slice(a * 128, b * 128)
                nc.sync.dma_start(out=w1a_sb[:, sl], in_=w1a_d[:, sl])
                if i % 2 == 0:
                    nc.vector.tensor_copy(w1a16[:, sl], w1a_sb[:, sl])
                else:
                    nc.scalar.copy(w1a16[:, sl], w1a_sb[:, sl])
            w1b_sb = pp.tile([128, KT1 * 64], F32, tag="w1b")
            w1b16 = pp.tile([128, KT1 * 64], F16, tag="w1b16")
            for i, (a, b) in enumerate(zip(W1B_CHUNKS[:-1], W1B_CHUNKS[1:])):
                sl = slice(a * 64, b * 64)
                nc.sync.dma_start(out=w1b_sb[:, sl], in_=w1b_d[:, sl])
                if i % 2 == 0:
                    nc.vector.tensor_copy(w1b16[:, sl], w1b_sb[:, sl])
                else:
                    nc.scalar.copy(w1b16[:, sl], w1b_sb[:, sl])

            # --- W2 shard (both m-blocks), then hpo M.T shard ---
            w2_sb = pp.tile([128, 2 * KT2 * 128], F32, tag="w2")
            w2_16 = pp.tile([128, 2 * KT2 * 128], F16, tag="w2_16")
            for mb in range(2):
                sl = slice(mb * KT2 * 128, (mb + 1) * KT2 * 128)
                nc.sync.dma_start(out=w2_sb[:, sl], in_=w2_d[:, sl])
                nc.vector.tensor_copy(w2_16[:, sl], w2_sb[:, sl])
            mt_sb = pp.tile([128, 2 * C], F32, tag="mt")
            cm_sb = sp.tile([128, 2], F32, tag="cm")
            for mb in range(2):
                sl = slice(mb * C, (mb + 1) * C)
                nc.sync.dma_start(out=mt_sb[:, sl], in_=mt_d[:, sl])
                nc.vector.reduce_max(
                    cm_sb[:, mb : mb + 1], mt_sb[:, sl], axis=mybir.AxisListType.X
                )

            # --- matmul1: h.T chunk = W1c.T @ gos.T, k accumulated in PSUM ---
            ph0 = psp.tile([128, B], F32, tag="ph0")
            for n in range(KT1):
                nc.tensor.matmul(
                    ph0[:, :],
                    lhsT=w1a16[:, n * 128 : (n + 1) * 128],
                    rhs=gos16[:, n * B : (n + 1) * B],
                    start=(n == 0),
                    stop=(n == KT1 - 1),
                )
            ph1 = psp.tile([64, B], F32, tag="ph1")
            for n in range(KT1):
                nc.tensor.matmul(
                    ph1[:, :],
                    lhsT=w1b16[:, n * 64 : (n + 1) * 64],
                    rhs=gos16[:, n * B : (n + 1) * B],
                    start=(n == 0),
                    stop=(n == KT1 - 1),
                )

            # gelu(pre + b1) (exact erf gelu), output fp16 for the gather
            h0_sb = sp.tile([128, B], F16, tag="h0")
            nc.scalar.activation(
                h0_sb[:, :], ph0[:, :],
                mybir.ActivationFunctionType.Gelu, bias=b1_sb[:, 0:1],
            )
            h1_sb = sp.tile([64, B], F16, tag="h1")
            nc.scalar.activation(
                h1_sb[:, :], ph1[:, :],
                mybir.ActivationFunctionType.Gelu, bias=b1_sb[0:64, 1:2],
            )

            # --- AllGather the (192, B) fp16 chunk -> full (1536, B) h.T ---
            ag_in = dp.tile([HD, B], F16, tag="ag_in")
            nc.sync.dma_start(out=ag_in[0:128, :], in_=h0_sb[:, :])
            nc.sync.dma_start(out=ag_in[128:HD, :], in_=h1_sb[:, :])
            ag_out = dp.tile([HIDP, B], F16, tag="ag_out")
            nc.gpsimd.collective_compute(
                "AllGather",
                mybir.AluOpType.bypass,
                replica_groups=[list(range(NCORES))],
                ins=[ag_in[:, :].opt()],
                outs=[ag_out[:, :].opt()],
            )

            # full x.T (k on partitions): 12 tiles from the gather + exp tile.
            # Split into 4 DMAs so the 128B-chunk strided loads run on
            # parallel queues.
            x_sb = pp.tile([128, (KT2 - 1) * B], F16, tag="xsb")
            for q in range(4):
                t0, t1 = 3 * q, 3 * (q + 1)
                nc.sync.dma_start(
                    out=x_sb[:, t0 * B : t1 * B].rearrange(
                        "p (n m) -> p n m", n=3
                    ),
                    in_=ag_out[t0 * 128 : t1 * 128, :].rearrange(
                        "(n p) m -> p n m", p=128
                    ),
                )

            # --- matmul2: flat.T chunk = W2c.T @ x.T ---
            pf0 = psp.tile([128, B], F32, tag="pf0")
            pf1 = psp.tile([128, B], F32, tag="pf1")
            for mb, pf in ((0, pf0), (1, pf1)):
                base = mb * KT2 * 128
                for t in range(KT2):
                    rhs = (
                        x_sb[:, t * B : (t + 1) * B]
                        if t < KT2 - 1
                        else exp16[:, :]
                    )
                    nc.tensor.matmul(
                        pf[:, :],
                        lhsT=w2_16[:, base + t * 128 : base + (t + 1) * 128],
                        rhs=rhs,
                        start=(t == 0),
                        stop=(t == KT2 - 1),
                    )

            # sigmoid(pre + b2)
            f0_sb = sp.tile([128, B], F32, tag="f0")
            nc.scalar.activation(
                f0_sb[:, :], pf0[:, :],
                mybir.ActivationFunctionType.Sigmoid, bias=b2_sb[:, 0:1],
            )
            f1_sb = sp.tile([128, B], F32, tag="f1")
            nc.scalar.activation(
                f1_sb[:, :], pf1[:, :],
                mybir.ActivationFunctionType.Sigmoid, bias=b2_sb[:, 1:2],
            )

            # out.T chunk = flat.T * colmax (per-partition broadcast)
            o_sb = sp.tile([128, 2 * B], F32, tag="osb")
            nc.vector.tensor_scalar_mul(o_sb[:, :B], f0_sb[:, :], cm_sb[:, 0:1])
            nc.vector.tensor_scalar_mul(o_sb[:, B:], f1_sb[:, :], cm_sb[:, 1:2])
            nc.sync.dma_start(out=out_d[:, :], in_=o_sb[:, :])

    nc.compile()
    return nc


_NC_CACHE = None


def _get_nc():
    global _NC_CACHE
    if _NC_CACHE is None:
        _NC_CACHE = _build_nc()
    return _NC_CACHE


def _tile_img(arr2d, ktiles):
    """(ktiles*128, m) -> SBUF image (128, ktiles*m), k-tile-major free dim."""
    k, m = arr2d.shape
    assert k == ktiles * 128
    return np.ascontiguousarray(
        arr2d.reshape(ktiles, 128, m).transpose(1, 0, 2).reshape(128, ktiles * m)
    )


def _prep_inputs(gos, exp_x, W1, b1, W2, b2, hpo_matrix):
    f = np.float32
    gos = np.asarray(gos, f)
    exp_x = np.asarray(exp_x, f)
    W1 = np.asarray(W1, f)
    b1 = np.asarray(b1, f)
    W2 = np.asarray(W2, f)
    b2 = np.asarray(b2, f)
    M = np.asarray(hpo_matrix, f)

    # gos.T padded to K1P rows, shared across cores
    gosT = np.zeros((K1P, B), f)
    gosT[:IN] = gos.T
    gos_img = _tile_img(gosT, KT1)

    # exp_x.T padded to one 128-row k-tile, shared
    expT = np.zeros((128, B), f)
    expT[:EXP] = exp_x.T
    exp_img = np.ascontiguousarray(expT)

    # W1 padded to (K1P, HIDP)
    W1p = np.zeros((K1P, HIDP), f)
    W1p[:IN, :HID] = W1
    b1p = np.zeros((HIDP,), f)
    b1p[:HID] = b1

    # W2 rows remapped to x.T layout [h(0:1536) | exp(1536:1589) | 0 pad]
    W2p = np.zeros((K2P, C), f)
    W2p[:HID] = W2[:HID]
    W2p[HIDP : HIDP + EXP] = W2[HID:]

    in_maps = []
    for c in range(NCORES):
        h0, h1 = HD * c, HD * (c + 1)
        c0, c1 = CD * c, CD * (c + 1)

        w1a_img = _tile_img(W1p[:, h0 : h0 + 128], KT1)
        w1b_img = _tile_img(W1p[:, h0 + 128 : h1], KT1)

        w2c = W2p[:, c0:c1]
        w2_img = np.concatenate(
            [_tile_img(w2c[:, :128], KT2), _tile_img(w2c[:, 128:], KT2)], axis=1
        )

        mt = np.ascontiguousarray(M[:, c0:c1].T)  # (256, 2048)
        mt_img = np.concatenate([mt[:128], mt[128:]], axis=1)  # (128, 2C)

        b1_img = np.zeros((128, 2), f)
        b1_img[:, 0] = b1p[h0 : h0 + 128]
        b1_img[:64, 1] = b1p[h0 + 128 : h1]
        b2_img = np.zeros((128, 2), f)
        b2_img[:, 0] = b2[c0 : c0 + 128]
        b2_img[:, 1] = b2[c0 + 128 : c1]

        in_maps.append(
            {
                "gos_img": gos_img,
                "w1a_img": np.ascontiguousarray(w1a_img),
                "w1b_img": np.ascontiguousarray(w1b_img),
                "w2_img": np.ascontiguousarray(w2_img),
                "exp_img": exp_img,
                "mt_img": np.ascontiguousarray(mt_img),
                "b1_img": b1_img,
                "b2_img": b2_img,
            }
        )
    return in_maps


def _assemble_output(results):
    cols = []
    for r in results:
        o = r["out_img"]  # (128, 2B): [p, t*B + b] = flat.T[t*128+p, b] * cm
        chunk = o.reshape(128, 2, B).transpose(1, 0, 2).reshape(CD, B)
        cols.append(chunk.T)  # (B, CD)
    return np.ascontiguousarray(np.concatenate(cols, axis=1))


def kernel(gos, exp_x, W1, b1, W2, b2, hpo_matrix, **kw):
    nc = _get_nc()
    in_maps = _prep_inputs(gos, exp_x, W1, b1, W2, b2, hpo_matrix)
    res = run_bass_kernel_spmd(nc, in_maps, core_ids=list(range(NCORES)))
    return _assemble_output(res.results)
